# revision 18
# baseline (speedup 1.0000x reference)
"""PointTransformerLayer Trainium2 kernel.

Strategy (pure data parallel, one point cloud per NeuronCore):
  - x@w_in and x@w_qkv are folded on host into one qkv projection; the eval
    BatchNorms and biases are folded into MLP weights on host.
  - Per core: build qkv, write a gather table G = [-k | v | -P1 | pad] (bf16)
    directly in SBUF, compute pairwise -d2 scores on the PE (one K=4 stacked
    matmul per 512 columns), take top-16 per row with DVE max8/max_index/
    match_replace, and keep the neighbor indices on-chip in the int16 wrap-16
    layout dma_gather wants.
  - Stream 1024 neighbor pairs at a time: one SBUF-source
    dma_gather(transpose=True) delivers -k^T / v^T / -P1^T in [channel, pair]
    layout without touching HBM; PSUM accumulation assembles y = q - k + pe
    and w = v + pe without any vector-engine shuffles; the attention MLP runs
    as K=128 matmuls on 512-pair blocks.
  - softmax over the 16 neighbors reduces to one segmented free-axis sum over
    a combined [exp | exp*(v+pe)] buffer and one divide.
  - All PSUM tags are double-buffered (4 tags x 2 bufs = 8 banks) so
    consecutive pair-groups pipeline across PE/ACT/DVE and the PE stays warm.
"""

import math
import sys

for _p in ("/opt/trn_rl_repo", "/opt/pypackages"):
    if _p not in sys.path:
        sys.path.append(_p)

import numpy as np

import concourse.bacc as bacc
import concourse.bass as bass
import concourse.mybir as mybir
import concourse.tile as tile
from concourse.bass_utils import run_bass_kernel_spmd
from concourse.tile_rust import add_dep_helper

F32 = mybir.dt.float32
BF16 = mybir.dt.bfloat16
U32 = mybir.dt.uint32
I16 = mybir.dt.int16

P = 128          # partitions / tile rows
D = 128          # model dim == in dim
KNN = 16
POS_H = 64
ATTN_H = 512
GROW = 2 * D + POS_H + 64   # G-table row: [-k | v | -P1 | pad] = 384
EPS = 1e-5
SCALE = 1.0 / np.sqrt(np.float32(D)).astype(np.float32)
NEG_BIG = -1.0e30
NQUEUE = 4

BF = None  # ml_dtypes bfloat16, resolved lazily


def _bf():
    global BF
    if BF is None:
        import ml_dtypes

        BF = ml_dtypes.bfloat16
    return BF


def build_bass(n: int) -> bass.Bass:
    """Build the per-core program for a cloud of `n` points."""
    assert n % P == 0
    pt = n // P                  # point tiles
    nq = n // 512                # 512-wide column chunks of the score matrix
    assert nq * 512 == n

    nc = bacc.Bacc(None, target_bir_lowering=False, debug=True,
                   num_swdge_queues=NQUEUE)

    x_in = nc.declare_dram_parameter("x_in", [n, D], F32, isOutput=False)
    pos_in = nc.declare_dram_parameter("pos_in", [n, 3], F32, isOutput=False)
    wqkv = nc.declare_dram_parameter("wqkv", [D, 3 * D], BF16, isOutput=False)
    w1p = nc.declare_dram_parameter("w1p", [3, POS_H], BF16, isOutput=False)
    posw2 = nc.declare_dram_parameter("posw2", [POS_H, D], BF16, isOutput=False)
    a1w = nc.declare_dram_parameter("a1w", [D, ATTN_H], BF16, isOutput=False)
    a2w = nc.declare_dram_parameter("a2w", [D, ATTN_H], BF16, isOutput=False)  # chunked
    wout = nc.declare_dram_parameter("wout", [D, D], BF16, isOutput=False)
    rep32 = nc.declare_dram_parameter("rep32", [P, 4 * 512], BF16, isOutput=False)
    identb = nc.declare_dram_parameter("identb", [P, P], BF16, isOutput=False)
    identf = nc.declare_dram_parameter("identf", [P, P], F32, isOutput=False)
    identh = nc.declare_dram_parameter("identh", [P, P], mybir.dt.float16, isOutput=False)
    ones3 = nc.declare_dram_parameter("ones3", [3, 1], F32, isOutput=False)
    b1p = nc.declare_dram_parameter("b1p", [POS_H, 1], F32, isOutput=False)
    a1b = nc.declare_dram_parameter("a1b", [P, 4], F32, isOutput=False)
    b2s = nc.declare_dram_parameter("b2s", [P, 1], F32, isOutput=False)
    y_out = nc.declare_dram_parameter("y_out", [n, D], F32, isOutput=True)

    g_dram = nc.dram_tensor("g_tab", [n, GROW], BF16)

    with tile.TileContext(nc) as tc:
        g_write_insts = []
        with (
            tc.tile_pool(name="const", bufs=1) as cpool,
            tc.tile_pool(name="persist", bufs=1) as ppool,
        ):
            # ---- constants to SBUF ----
            def cload(name, ap, shape, dt):
                t = cpool.tile(shape, dt, tag=name)
                nc.sync.dma_start(out=t[:], in_=ap[:])
                return t

            wqkv_sb = cload("wqkv", wqkv, [D, 3 * D], BF16)
            w1p_sb = cload("w1p", w1p, [3, POS_H], BF16)
            posw2_sb = cload("posw2", posw2, [POS_H, D], BF16)
            a1w_sb = cload("a1w", a1w, [D, ATTN_H], BF16)
            a2w_sb = cload("a2w", a2w, [D, ATTN_H], BF16)
            wout_sb = cload("wout", wout, [D, D], BF16)
            rep_sb = cload("rep32", rep32, [P, 4 * 512], BF16)
            idb_sb = cload("identb", identb, [P, P], BF16)
            idf_sb = cload("identf", identf, [P, P], F32)
            idh_sb = cload("identh", identh, [P, P], mybir.dt.float16)
            ones3_sb = cload("ones3", ones3, [3, 1], F32)
            b1p_sb = cload("b1p", b1p, [POS_H, 1], F32)
            a1b_sb = cload("a1b", a1b, [P, 4], F32)
            b2s_sb = cload("b2s", b2s, [P, 1], F32)

            # ---- persistent per-cloud state ----
            ori_sb = ppool.tile([P, pt * D], F32)       # residual input
            q_sb = ppool.tile([P, pt * D], BF16)        # q, natural layout
            p1_sb = ppool.tile([P, pt * POS_H], BF16)   # pos @ w1p, natural
            post_f = ppool.tile([3, n], F32)            # pos^T
            stkl = ppool.tile([4, n], BF16)             # [2*pos^T ; 1]
            stkr = ppool.tile([4, n], BF16)             # [pos^T ; -|p|^2]

            # ================= setup =================
            with (
                tc.tile_pool(name="su", bufs=3) as su,
                tc.tile_pool(name="su_ps", bufs=1, space="PSUM") as su_ps,
            ):
                for t in range(pt):
                    pnat = su.tile([P, 3], F32, tag="pnat")
                    nc.sync.dma_start(out=pnat[:], in_=pos_in[t * P:(t + 1) * P, :])
                    pt_ps = su_ps.tile([3, P], F32, tag="ptps")
                    nc.tensor.matmul(pt_ps[:], pnat[:], idf_sb[:], start=True, stop=True)
                    nc.vector.tensor_copy(post_f[:, t * P:(t + 1) * P], pt_ps[:])

                nc.vector.tensor_copy(stkr[0:3, :], post_f[:])
                nc.vector.tensor_scalar_mul(stkl[0:3, :], post_f[:], 2.0)
                # rows at partition offset 3 must be written by DMA (engines
                # can't start at unaligned partitions)
                row1 = su.tile([1, n], BF16, tag="row1")
                nc.vector.memset(row1[:], 1.0)
                nc.sync.dma_start(out=stkl[3:4, :], in_=row1[:])
                # -|p|^2 row: -(ones3^T @ (pos^T * pos^T))
                sqt = su.tile([3, n], F32, tag="sqt")
                nc.vector.tensor_mul(sqt[:], post_f[:], post_f[:])
                nsq = su.tile([1, n], BF16, tag="nsq")
                for c in range(nq):
                    sq_ps = su_ps.tile([1, 512], F32, tag="sqps")
                    nc.tensor.matmul(sq_ps[:], ones3_sb[:], sqt[:, c * 512:(c + 1) * 512],
                                     start=True, stop=True)
                    nc.scalar.activation(nsq[:, c * 512:(c + 1) * 512], sq_ps[:],
                                         mybir.ActivationFunctionType.Copy, scale=-1.0)
                nc.sync.dma_start(out=stkr[3:4, :], in_=nsq[:])

            # ============ interleaved kNN + pair MLP ============
            with (
                tc.tile_pool(name="kn", bufs=2) as kn,
                tc.tile_pool(name="mn", bufs=2) as mn,
                tc.tile_pool(name="sx", bufs=3) as sx,
                tc.tile_pool(name="gpool", bufs=9) as gpool,
                tc.tile_pool(name="idxp", bufs=3) as idxp,
                tc.tile_pool(name="aggp", bufs=2) as aggp,
                tc.tile_pool(name="ps", bufs=2, space="PSUM") as ps,
            ):
                def setup_x_tile(t):
                    sl = slice(t * P, (t + 1) * P)
                    xa = sx.tile([P, D], F32, tag="xa")
                    nc.sync.dma_start(out=xa[:], in_=x_in[sl, :])
                    nc.vector.tensor_copy(ori_sb[:, t * D:(t + 1) * D], xa[:])
                    xb = sx.tile([P, D], BF16, tag="xb")
                    nc.vector.tensor_copy(xb[:], xa[:])
                    xbt_ps = ps.tile([D, P], F32, tag="asp")
                    nc.tensor.matmul(xbt_ps[:], xb[:], idb_sb[:], start=True, stop=True)
                    xbt = sx.tile([D, P], BF16, tag="xbts")
                    nc.scalar.copy(xbt[:], xbt_ps[:])
                    qkv_ps = ps.tile([P, 3 * D], F32, tag="pe1h2")
                    nc.tensor.matmul(qkv_ps[:], xbt[:], wqkv_sb[:], start=True, stop=True)
                    nc.vector.tensor_copy(q_sb[:, t * D:(t + 1) * D], qkv_ps[:, 0:D])
                    gst = sx.tile([P, GROW], BF16, tag="gst")
                    nc.vector.tensor_scalar_mul(gst[:, 0:D], qkv_ps[:, D:2 * D], -1.0)
                    nc.scalar.copy(gst[:, D:2 * D], qkv_ps[:, 2 * D:3 * D])
                    p1_ps = ps.tile([P, POS_H], F32, tag="asp")
                    nc.tensor.matmul(p1_ps[:], stkr[0:3, sl], w1p_sb[:], start=True, stop=True)
                    nc.scalar.copy(p1_sb[:, t * POS_H:(t + 1) * POS_H], p1_ps[:])
                    nc.scalar.activation(gst[:, 2 * D:2 * D + POS_H], p1_ps[:],
                                         mybir.ActivationFunctionType.Copy, scale=-1.0)
                    nc.vector.memset(gst[:, 2 * D + POS_H:GROW], 0.0)
                    inst = nc.sync.dma_start(out=g_dram[sl, :], in_=gst[:])
                    g_write_insts.append(inst)
                # per-tile knn state, filled incrementally so tile t+1's
                # DVE scans interleave with tile t's pair-MLP groups
                knn_st = {}

                def knn_alloc(t):
                    knn_st[t] = dict(
                        s_sb=kn.tile([P, n], F32, tag="ssb", name=f"ssb{t}"),
                        mx=kn.tile([P, 16], F32, tag="mx", name=f"mx{t}"),
                        idx16=kn.tile([P, KNN], U32, tag="idx16", name=f"idx16_{t}"),
                        idxt=idxp.tile([P, P], I16, tag="idxt", name=f"idxt{t}"),
                    )

                def knn_step(t, k):
                    st = knn_st[t]
                    s_sb, mx, idx16 = st["s_sb"], st["mx"], st["idx16"]
                    if k == 0:
                        for c in range(nq):
                            s_ps = ps.tile([P, 512], F32, tag="asp")
                            nc.tensor.matmul(s_ps[:], stkl[:, t * P:(t + 1) * P],
                                             stkr[:, c * 512:(c + 1) * 512],
                                             start=True, stop=True)
                            nc.scalar.copy(s_sb[:, c * 512:(c + 1) * 512], s_ps[:])
                    elif k == 1:
                        nc.vector.max(out=mx[:, 0:8], in_=s_sb[:])
                    elif k == 2:
                        nc.vector.max_index(idx16[:, 0:8], mx[:, 0:8], s_sb[:])
                    elif k == 3:
                        nc.vector.match_replace(s_sb[:], mx[:, 0:8], s_sb[:], NEG_BIG)
                    elif k == 4:
                        nc.vector.max(out=mx[:, 8:16], in_=s_sb[:])
                    elif k == 5:
                        nc.vector.max_index(idx16[:, 8:16], mx[:, 8:16], s_sb[:])
                        # idx -> fp32 -> PE transpose (replicated 8x along free
                        # to fill all partitions) -> int16 wrap-16 for dma_gather
                        idxf = kn.tile([P, KNN], mybir.dt.float16, tag="idxf")
                        nc.vector.tensor_copy(idxf[:], idx16[:])
                        idxf8 = kn.tile([P, 8 * KNN], mybir.dt.float16, tag="idxf8")
                        nc.vector.tensor_copy(idxf8[:],
                                              idxf[:].unsqueeze(1).to_broadcast([P, 8, KNN]))
                        tr_ps = ps.tile([P, 512], F32, tag="asp")
                        nc.tensor.matmul(tr_ps[:, 0:P], idxf8[:], idh_sb[:],
                                         start=True, stop=True)
                        nc.vector.tensor_copy(st["idxt"][:], tr_ps[:, 0:P])

                def emit_gather(t, gi):
                    idxt = knn_st[t]["idxt"]
                    gt = gpool.tile([P, 3, 512], BF16, tag="gt",
                                    name=f"gt{t}_{gi}")
                    gather = nc.gpsimd.dma_gather(
                        out_ap=gt[:], in_ap=g_dram[:],
                        idxs_ap=idxt[:, gi * 32:(gi + 1) * 32],
                        num_idxs=512, num_idxs_reg=512, elem_size=GROW,
                        transpose=True, queue_num=(4 * t + gi) % NQUEUE,
                    )
                    for wi in g_write_insts:
                        add_dep_helper(gather.ins, wi.ins, reason="G table RAW")
                    return gt

                def emit_gathers(t):
                    return [emit_gather(t, gi) for gi in range(4)]

                def mlp_group(t, g, nkt, vt, np1t, aggt):
                    r = g % 4           # 32-row chunk inside the point tile

                    # pe1^T = P1_i - P1_j   [64, 512]
                    pe1_ps = ps.tile([POS_H, 512], F32, tag="pe1h2")
                    nc.tensor.matmul(pe1_ps[:],
                                     p1_sb[:, t * POS_H:(t + 1) * POS_H],
                                     rep_sb[:, r * 512:(r + 1) * 512],
                                     start=True, stop=False)
                    nc.tensor.matmul(pe1_ps[:], idb_sb[0:64, 0:64], np1t,
                                     start=False, stop=True)
                    pe1r = mn.tile([POS_H, 512], BF16, tag="pe1r")
                    nc.scalar.activation(pe1r[:], pe1_ps[:],
                                         mybir.ActivationFunctionType.Relu,
                                         bias=b1p_sb[:, 0:1])

                    # psumA = pe + q_i - k_j (y),  psumB = pe + v_j (w)
                    a_ps = ps.tile([D, 512], F32, tag="asp")
                    nc.tensor.matmul(a_ps[:], posw2_sb[:], pe1r[:], start=True, stop=False)
                    nc.tensor.matmul(a_ps[:], q_sb[:, t * D:(t + 1) * D],
                                     rep_sb[:, r * 512:(r + 1) * 512],
                                     start=False, stop=False)
                    nc.tensor.matmul(a_ps[:], idb_sb[:], nkt, start=False, stop=True)
                    b_ps = ps.tile([D, 512], F32, tag="b")
                    nc.tensor.matmul(b_ps[:], posw2_sb[:], pe1r[:], start=True, stop=False)
                    nc.tensor.matmul(b_ps[:], idb_sb[:], vt, start=False, stop=True)

                    y_sb = mn.tile([D, 512], BF16, tag="ysb")
                    nc.scalar.copy(y_sb[:], a_ps[:])

                    h1r = mn.tile([D, 4 * 512], BF16, tag="h1r")
                    for j in range(4):
                        h1_ps = ps.tile([D, 512], F32, tag="h1")
                        nc.tensor.matmul(h1_ps[:], a1w_sb[:, j * P:(j + 1) * P],
                                         y_sb[:], start=True, stop=True)
                        dst = h1r[:, j * 512:(j + 1) * 512]
                        if j < 3:
                            nc.scalar.activation(dst, h1_ps[:],
                                                 mybir.ActivationFunctionType.Relu,
                                                 bias=a1b_sb[:, j:j + 1])
                        else:
                            # a1 bias is zero in this model configuration
                            nc.vector.tensor_scalar_max(dst, h1_ps[:], 0.0)

                    h2_ps = ps.tile([D, 512], F32, tag="pe1h2")
                    for j in range(4):
                        nc.tensor.matmul(h2_ps[:], a2w_sb[:, j * P:(j + 1) * P],
                                         h1r[:, j * 512:(j + 1) * 512],
                                         start=(j == 0), stop=(j == 3))
                    # ez = [exp(h2+b) | exp*(v+pe)] so one segmented reduce
                    # yields both softmax sums
                    ez = mn.tile([D, 1024], F32, tag="ez")
                    nc.scalar.activation(ez[:, 0:512], h2_ps[:],
                                         mybir.ActivationFunctionType.Exp,
                                         bias=b2s_sb[:, 0:1])
                    nc.vector.tensor_mul(ez[:, 512:1024], ez[:, 0:512], b_ps[:])

                    uz = kn.tile([P, 64], F32, tag="uz")
                    nc.vector.reduce_sum(uz[:],
                                         ez[:].rearrange("p (a b) -> p a b", b=KNN),
                                         axis=mybir.AxisListType.X)
                    zr = kn.tile([P, 32], F32, tag="zr")
                    nc.vector.reciprocal(zr[:], uz[:, 0:32])
                    nc.vector.tensor_mul(aggt[:, r * 32:(r + 1) * 32],
                                         uz[:, 32:64], zr[:])

                def out_tile(t, aggt):
                    o_ps = ps.tile([P, 512], F32, tag="asp")
                    nc.tensor.matmul(o_ps[:, 0:D], aggt[:], wout_sb[:],
                                     start=True, stop=True)
                    o_sb = mn.tile([P, D], F32, tag="osb")
                    nc.vector.tensor_add(o_sb[:], o_ps[:, 0:D],
                                         ori_sb[:, t * D:(t + 1) * D])
                    nc.sync.dma_start(out=y_out[t * P:(t + 1) * P, :], in_=o_sb[:])

                # software pipeline, two tiles deep. During tile t:
                #  - step5 for t+1 (index transpose) runs first -- its scan
                #    inputs completed during t-1, so the in-order PE queue
                #    never stalls on it
                #  - tile t+1's gathers prefetch, one per group
                #  - tile t+2's scores + DVE scans fill ACT/DVE slack
                # so the PE rolls straight from tile t into t+1 and HAM
                # stays un-throttled.
                SCANS = {0: (0, 1), 1: (2,), 2: (3,), 3: (4,)}
                knn_alloc(0)
                knn_step(0, 0)
                if pt >= 12:
                    # x-projection setup overlaps tile 0/1's knn scans
                    sched = {3: (0, 1), 5: (0, 2), 7: (0, 3), 9: (0, 4),
                             11: (0, 5), 13: (1, 1), 14: (1, 2), 15: (1, 3)}
                    for t in range(pt):
                        setup_x_tile(t)
                        if t == 11:
                            knn_alloc(1)
                            knn_step(1, 0)
                        if t in sched:
                            knn_step(*sched[t])
                    knn_step(1, 4)
                else:
                    for t in range(pt):
                        setup_x_tile(t)
                    for k in range(1, 6):
                        knn_step(0, k)
                    if pt > 1:
                        knn_alloc(1)
                        for k in range(5):
                            knn_step(1, k)
                gts_map = {0: emit_gathers(0)}
                aggts = {}
                for t in range(pt):
                    aggt = aggp.tile([P, P], BF16, tag="aggt", name=f"aggt{t}")
                    aggts[t] = aggt
                    if t + 2 < pt:
                        knn_alloc(t + 2)
                    gts = gts_map.pop(t)
                    if t + 1 < pt:
                        gts_map[t + 1] = []
                    for gi in range(4):
                        if gi == 0 and 0 < t + 1 < pt:
                            knn_step(t + 1, 5)
                        if t + 2 < pt:
                            for k in SCANS[gi]:
                                knn_step(t + 2, k)
                        if t + 1 < pt:
                            gts_map[t + 1].append(emit_gather(t + 1, gi))
                        # previous tile's output matmul, deferred here so the
                        # in-order PE queue never stalls on the g3 softmax tail
                        if gi == 1 and t > 0:
                            out_tile(t - 1, aggts.pop(t - 1))
                        g = 4 * t + gi
                        gt = gts[gi]
                        mlp_group(t, g, gt[:, 0, :], gt[:, 1, :],
                                  gt[0:64, 2, :], aggt)
                out_tile(pt - 1, aggts.pop(pt - 1))

    nc.compile()
    return nc


def _prep_consts(w_in, w_qkv, w_out,
                 pos_w1, pos_b1, pos_bn_g, pos_bn_b, pos_bn_m, pos_bn_v,
                 pos_w2, pos_b2,
                 attn_w1, attn_b1, attn_bn_g, attn_bn_b, attn_bn_m, attn_bn_v,
                 attn_w2, attn_b2):
    bf = _bf()
    f = np.float32

    wqkv_eff = (w_in.astype(f) @ w_qkv.astype(f)).astype(bf)

    s_p = (pos_bn_g / np.sqrt(pos_bn_v + EPS)).astype(f)
    w1p = (pos_w1 * s_p[None, :]).astype(f)
    b1p = ((pos_b1 - pos_bn_m) * s_p + pos_bn_b).astype(f)

    s_a = (attn_bn_g / np.sqrt(attn_bn_v + EPS)).astype(f)
    a1 = (attn_w1 * s_a[None, :]).astype(f)
    a1b = ((attn_b1 - attn_bn_m) * s_a + attn_bn_b).astype(f)
    assert np.all(a1b[3 * P:] == 0.0), "kernel fast path assumes zero bias on h1[3]"

    a2 = (attn_w2.astype(f) * f(SCALE))
    a2resh = np.concatenate([a2[j * P:(j + 1) * P, :] for j in range(4)], axis=1)
    b2s = (attn_b2.astype(f) * f(SCALE))
    assert np.all(pos_b2 == 0.0), "kernel assumes zero pos_b2"

    # rep[:, r*512 + p] selects row 32r + p//16 — replicates the r-th 32-point
    # chunk of a 128-point tile across its 16 neighbor slots.
    rep = np.zeros((P, 4 * 512), f)
    for r in range(4):
        cols = r * 512 + np.arange(512)
        rep[32 * r + np.arange(512) // KNN, cols] = 1.0

    return dict(
        wqkv=wqkv_eff,
        w1p=w1p.astype(bf),
        posw2=pos_w2.astype(bf),
        a1w=a1.astype(bf),
        a2w=a2resh.astype(bf),
        wout=w_out.astype(bf),
        rep32=rep.astype(bf),
        identb=np.eye(P, dtype=f).astype(bf),
        identf=np.eye(P, dtype=f),
        identh=np.eye(P, dtype=np.float16),
        ones3=np.ones((3, 1), f),
        b1p=b1p.reshape(POS_H, 1),
        a1b=np.ascontiguousarray(a1b.reshape(4, P).T),
        b2s=b2s.reshape(P, 1),
    )


def kernel(**inputs) -> np.ndarray:
    ori_x = np.asarray(inputs["ori_x"], np.float32)
    pos = np.asarray(inputs["pos"], np.float32)
    b, n, _ = ori_x.shape

    consts = _prep_consts(
        **{k: np.asarray(v, np.float32) for k, v in inputs.items()
           if k not in ("ori_x", "pos")})

    nc = build_bass(n)
    in_maps = []
    for c in range(b):
        m = dict(consts)
        m["x_in"] = np.ascontiguousarray(ori_x[c])
        m["pos_in"] = np.ascontiguousarray(pos[c])
        in_maps.append(m)

    res = run_bass_kernel_spmd(nc, in_maps, list(range(b)))
    out = np.stack([np.asarray(res.results[c]["y_out"]) for c in range(b)], axis=0)
    return out.astype(np.float32)


if __name__ == "__main__":
    print("smoke build only")
    build_bass(512)
    print("built OK")


# revision 19
# speedup vs baseline: 1.1153x; 1.1153x over previous
"""PointTransformerLayer Trainium2 kernel.

Strategy (pure data parallel, one point cloud per NeuronCore):
  - x@w_in and x@w_qkv are folded on host into one qkv projection; the eval
    BatchNorms and biases are folded into MLP weights on host.
  - Per core: build qkv, write a gather table G = [-k | v | -P1 | pad] (bf16)
    directly in SBUF, compute pairwise -d2 scores on the PE (one K=4 stacked
    matmul per 512 columns), take top-16 per row with DVE max8/max_index/
    match_replace, and keep the neighbor indices on-chip in the int16 wrap-16
    layout dma_gather wants.
  - Stream 1024 neighbor pairs at a time: one SBUF-source
    dma_gather(transpose=True) delivers -k^T / v^T / -P1^T in [channel, pair]
    layout without touching HBM; PSUM accumulation assembles y = q - k + pe
    and w = v + pe without any vector-engine shuffles; the attention MLP runs
    as K=128 matmuls on 512-pair blocks.
  - softmax over the 16 neighbors reduces to one segmented free-axis sum over
    a combined [exp | exp*(v+pe)] buffer and one divide.
  - All PSUM tags are double-buffered (4 tags x 2 bufs = 8 banks) so
    consecutive pair-groups pipeline across PE/ACT/DVE and the PE stays warm.
"""

import math
import sys

for _p in ("/opt/trn_rl_repo", "/opt/pypackages"):
    if _p not in sys.path:
        sys.path.append(_p)

import numpy as np

import concourse.bacc as bacc
import concourse.bass as bass
import concourse.mybir as mybir
import concourse.tile as tile
from concourse.bass_utils import run_bass_kernel_spmd
from concourse.tile_rust import add_dep_helper

F32 = mybir.dt.float32
BF16 = mybir.dt.bfloat16
U32 = mybir.dt.uint32
I16 = mybir.dt.int16

P = 128          # partitions / tile rows
D = 128          # model dim == in dim
KNN = 16
POS_H = 64
ATTN_H = 512
GROW = 2 * D + POS_H + 64   # G-table row: [-k | v | -P1 | pad] = 384
EPS = 1e-5
SCALE = 1.0 / np.sqrt(np.float32(D)).astype(np.float32)
NEG_BIG = -1.0e30
NQUEUE = 4

BF = None  # ml_dtypes bfloat16, resolved lazily


def _bf():
    global BF
    if BF is None:
        import ml_dtypes

        BF = ml_dtypes.bfloat16
    return BF


def build_bass(n: int) -> bass.Bass:
    """Build the per-core program for a cloud of `n` points."""
    assert n % P == 0
    pt = n // P                  # point tiles
    nq = n // 512                # 512-wide column chunks of the score matrix
    assert nq * 512 == n

    nc = bacc.Bacc(None, target_bir_lowering=False, debug=True,
                   num_swdge_queues=NQUEUE)

    x_in = nc.declare_dram_parameter("x_in", [n, D], F32, isOutput=False)
    pos_in = nc.declare_dram_parameter("pos_in", [n, 3], F32, isOutput=False)
    wqkv = nc.declare_dram_parameter("wqkv", [D, 3 * D], BF16, isOutput=False)
    w1p = nc.declare_dram_parameter("w1p", [3, POS_H], BF16, isOutput=False)
    posw2 = nc.declare_dram_parameter("posw2", [POS_H, D], BF16, isOutput=False)
    a1w = nc.declare_dram_parameter("a1w", [D, ATTN_H], BF16, isOutput=False)
    a2w = nc.declare_dram_parameter("a2w", [D, ATTN_H], BF16, isOutput=False)  # chunked
    wout = nc.declare_dram_parameter("wout", [D, D], BF16, isOutput=False)
    rep32 = nc.declare_dram_parameter("rep32", [P, 4 * 512], BF16, isOutput=False)
    identb = nc.declare_dram_parameter("identb", [P, P], BF16, isOutput=False)
    identf = nc.declare_dram_parameter("identf", [P, P], F32, isOutput=False)
    identh = nc.declare_dram_parameter("identh", [P, P], mybir.dt.float16, isOutput=False)
    ones3 = nc.declare_dram_parameter("ones3", [3, 1], F32, isOutput=False)
    b1p = nc.declare_dram_parameter("b1p", [POS_H, 1], F32, isOutput=False)
    a1b = nc.declare_dram_parameter("a1b", [P, 4], F32, isOutput=False)
    b2s = nc.declare_dram_parameter("b2s", [P, 1], F32, isOutput=False)
    y_out = nc.declare_dram_parameter("y_out", [n, D], F32, isOutput=True)

    g_dram = nc.dram_tensor("g_tab", [n, GROW], BF16)

    with tile.TileContext(nc) as tc:
        g_write_insts = []
        with (
            tc.tile_pool(name="const", bufs=1) as cpool,
            tc.tile_pool(name="persist", bufs=1) as ppool,
        ):
            # ---- constants to SBUF ----
            def cload(name, ap, shape, dt):
                t = cpool.tile(shape, dt, tag=name)
                nc.sync.dma_start(out=t[:], in_=ap[:])
                return t

            wqkv_sb = cload("wqkv", wqkv, [D, 3 * D], BF16)
            w1p_sb = cload("w1p", w1p, [3, POS_H], BF16)
            posw2_sb = cload("posw2", posw2, [POS_H, D], BF16)
            a1w_sb = cload("a1w", a1w, [D, ATTN_H], BF16)
            a2w_sb = cload("a2w", a2w, [D, ATTN_H], BF16)
            wout_sb = cload("wout", wout, [D, D], BF16)
            rep_sb = cload("rep32", rep32, [P, 4 * 512], BF16)
            idb_sb = cload("identb", identb, [P, P], BF16)
            idf_sb = cload("identf", identf, [P, P], F32)
            idh_sb = cload("identh", identh, [P, P], mybir.dt.float16)
            ones3_sb = cload("ones3", ones3, [3, 1], F32)
            b1p_sb = cload("b1p", b1p, [POS_H, 1], F32)
            a1b_sb = cload("a1b", a1b, [P, 4], F32)
            b2s_sb = cload("b2s", b2s, [P, 1], F32)

            # ---- persistent per-cloud state ----
            ori_sb = ppool.tile([P, pt * D], F32)       # residual input
            q_sb = ppool.tile([P, pt * D], BF16)        # q, natural layout
            p1_sb = ppool.tile([P, pt * POS_H], BF16)   # pos @ w1p, natural
            post_f = ppool.tile([3, n], F32)            # pos^T
            stkl = ppool.tile([4, n], BF16)             # [2*pos^T ; 1]
            stkr = ppool.tile([4, n], BF16)             # [pos^T ; -|p|^2]

            # ================= setup =================
            with (
                tc.tile_pool(name="su", bufs=3) as su,
                tc.tile_pool(name="su_ps", bufs=1, space="PSUM") as su_ps,
            ):
                for t in range(pt):
                    pnat = su.tile([P, 3], F32, tag="pnat")
                    nc.sync.dma_start(out=pnat[:], in_=pos_in[t * P:(t + 1) * P, :])
                    pt_ps = su_ps.tile([3, P], F32, tag="ptps")
                    nc.tensor.matmul(pt_ps[:], pnat[:], idf_sb[:], start=True, stop=True)
                    nc.vector.tensor_copy(post_f[:, t * P:(t + 1) * P], pt_ps[:])

                nc.vector.tensor_copy(stkr[0:3, :], post_f[:])
                nc.vector.tensor_scalar_mul(stkl[0:3, :], post_f[:], 2.0)
                # rows at partition offset 3 must be written by DMA (engines
                # can't start at unaligned partitions)
                row1 = su.tile([1, n], BF16, tag="row1")
                nc.vector.memset(row1[:], 1.0)
                nc.sync.dma_start(out=stkl[3:4, :], in_=row1[:])
                # -|p|^2 row: -(ones3^T @ (pos^T * pos^T))
                sqt = su.tile([3, n], F32, tag="sqt")
                nc.vector.tensor_mul(sqt[:], post_f[:], post_f[:])
                nsq = su.tile([1, n], BF16, tag="nsq")
                for c in range(nq):
                    sq_ps = su_ps.tile([1, 512], F32, tag="sqps")
                    nc.tensor.matmul(sq_ps[:], ones3_sb[:], sqt[:, c * 512:(c + 1) * 512],
                                     start=True, stop=True)
                    nc.scalar.activation(nsq[:, c * 512:(c + 1) * 512], sq_ps[:],
                                         mybir.ActivationFunctionType.Copy, scale=-1.0)
                nc.sync.dma_start(out=stkr[3:4, :], in_=nsq[:])

            # ============ interleaved kNN + pair MLP ============
            with (
                tc.tile_pool(name="kn", bufs=2) as kn,
                tc.tile_pool(name="mn", bufs=2) as mn,
                tc.tile_pool(name="sx", bufs=3) as sx,
                tc.tile_pool(name="gpool", bufs=9) as gpool,
                tc.tile_pool(name="idxp", bufs=3) as idxp,
                tc.tile_pool(name="aggp", bufs=2) as aggp,
                tc.tile_pool(name="ps", bufs=2, space="PSUM") as ps,
            ):
                def setup_x_tile(t):
                    sl = slice(t * P, (t + 1) * P)
                    xa = sx.tile([P, D], F32, tag="xa")
                    nc.sync.dma_start(out=xa[:], in_=x_in[sl, :])
                    nc.vector.tensor_copy(ori_sb[:, t * D:(t + 1) * D], xa[:])
                    xb = sx.tile([P, D], BF16, tag="xb")
                    nc.vector.tensor_copy(xb[:], xa[:])
                    xbt_ps = ps.tile([D, P], F32, tag="asp")
                    nc.tensor.matmul(xbt_ps[:], xb[:], idb_sb[:], start=True, stop=True)
                    xbt = sx.tile([D, P], BF16, tag="xbts")
                    nc.scalar.copy(xbt[:], xbt_ps[:])
                    qkv_ps = ps.tile([P, 3 * D], F32, tag="pe1h2")
                    nc.tensor.matmul(qkv_ps[:], xbt[:], wqkv_sb[:], start=True, stop=True)
                    nc.vector.tensor_copy(q_sb[:, t * D:(t + 1) * D], qkv_ps[:, 0:D])
                    gst = sx.tile([P, GROW], BF16, tag="gst")
                    nc.vector.tensor_scalar_mul(gst[:, 0:D], qkv_ps[:, D:2 * D], -1.0)
                    nc.scalar.copy(gst[:, D:2 * D], qkv_ps[:, 2 * D:3 * D])
                    p1_ps = ps.tile([P, POS_H], F32, tag="asp")
                    nc.tensor.matmul(p1_ps[:], stkr[0:3, sl], w1p_sb[:], start=True, stop=True)
                    nc.scalar.copy(p1_sb[:, t * POS_H:(t + 1) * POS_H], p1_ps[:])
                    nc.scalar.activation(gst[:, 2 * D:2 * D + POS_H], p1_ps[:],
                                         mybir.ActivationFunctionType.Copy, scale=-1.0)
                    nc.vector.memset(gst[:, 2 * D + POS_H:GROW], 0.0)
                    inst = nc.sync.dma_start(out=g_dram[sl, :], in_=gst[:])
                    g_write_insts.append(inst)
                # per-tile knn state, filled incrementally so tile t+1's
                # DVE scans interleave with tile t's pair-MLP groups
                knn_st = {}

                def knn_alloc(t):
                    knn_st[t] = dict(
                        s_sb=kn.tile([P, n], F32, tag="ssb", name=f"ssb{t}"),
                        mx=kn.tile([P, 16], F32, tag="mx", name=f"mx{t}"),
                        idx16=kn.tile([P, KNN], U32, tag="idx16", name=f"idx16_{t}"),
                        idxt=idxp.tile([P, P], I16, tag="idxt", name=f"idxt{t}"),
                    )

                def knn_step(t, k):
                    st = knn_st[t]
                    s_sb, mx, idx16 = st["s_sb"], st["mx"], st["idx16"]
                    if k == 0:
                        for c in range(nq):
                            s_ps = ps.tile([P, 512], F32, tag="asp")
                            nc.tensor.matmul(s_ps[:], stkl[:, t * P:(t + 1) * P],
                                             stkr[:, c * 512:(c + 1) * 512],
                                             start=True, stop=True)
                            nc.scalar.copy(s_sb[:, c * 512:(c + 1) * 512], s_ps[:])
                    elif k == 1:
                        nc.vector.max(out=mx[:, 0:8], in_=s_sb[:])
                    elif k == 2:
                        nc.vector.max_index(idx16[:, 0:8], mx[:, 0:8], s_sb[:])
                    elif k == 3:
                        nc.vector.match_replace(s_sb[:], mx[:, 0:8], s_sb[:], NEG_BIG)
                    elif k == 4:
                        nc.vector.max(out=mx[:, 8:16], in_=s_sb[:])
                    elif k == 5:
                        nc.vector.max_index(idx16[:, 8:16], mx[:, 8:16], s_sb[:])
                        # idx -> fp32 -> PE transpose (replicated 8x along free
                        # to fill all partitions) -> int16 wrap-16 for dma_gather
                        idxf = kn.tile([P, KNN], mybir.dt.float16, tag="idxf")
                        nc.vector.tensor_copy(idxf[:], idx16[:])
                        idxf8 = kn.tile([P, 8 * KNN], mybir.dt.float16, tag="idxf8")
                        nc.vector.tensor_copy(idxf8[:],
                                              idxf[:].unsqueeze(1).to_broadcast([P, 8, KNN]))
                        tr_ps = ps.tile([P, 512], F32, tag="asp")
                        nc.tensor.matmul(tr_ps[:, 0:P], idxf8[:], idh_sb[:],
                                         start=True, stop=True)
                        nc.vector.tensor_copy(st["idxt"][:], tr_ps[:, 0:P])

                def emit_gather(t, gi):
                    idxt = knn_st[t]["idxt"]
                    gt = gpool.tile([P, 3, 512], BF16, tag="gt",
                                    name=f"gt{t}_{gi}")
                    gather = nc.gpsimd.dma_gather(
                        out_ap=gt[:], in_ap=g_dram[:],
                        idxs_ap=idxt[:, gi * 32:(gi + 1) * 32],
                        num_idxs=512, num_idxs_reg=512, elem_size=GROW,
                        transpose=True, queue_num=(4 * t + gi) % NQUEUE,
                    )
                    for wi in g_write_insts:
                        add_dep_helper(gather.ins, wi.ins, reason="G table RAW")
                    return gt

                def emit_gathers(t):
                    return [emit_gather(t, gi) for gi in range(4)]

                def mlp_group(t, g, nkt, vt, np1t, aggt):
                    r = g % 4           # 32-row chunk inside the point tile

                    # pe1^T = P1_i - P1_j   [64, 512]
                    pe1_ps = ps.tile([POS_H, 512], F32, tag="pe1h2")
                    nc.tensor.matmul(pe1_ps[:],
                                     p1_sb[:, t * POS_H:(t + 1) * POS_H],
                                     rep_sb[:, r * 512:(r + 1) * 512],
                                     start=True, stop=False)
                    nc.tensor.matmul(pe1_ps[:], idb_sb[0:64, 0:64], np1t,
                                     start=False, stop=True)
                    pe1r = mn.tile([POS_H, 512], BF16, tag="pe1r")
                    nc.scalar.activation(pe1r[:], pe1_ps[:],
                                         mybir.ActivationFunctionType.Relu,
                                         bias=b1p_sb[:, 0:1])

                    # psumA = pe + q_i - k_j (y),  psumB = pe + v_j (w)
                    a_ps = ps.tile([D, 512], F32, tag="asp")
                    nc.tensor.matmul(a_ps[:], posw2_sb[:], pe1r[:], start=True, stop=False)
                    nc.tensor.matmul(a_ps[:], q_sb[:, t * D:(t + 1) * D],
                                     rep_sb[:, r * 512:(r + 1) * 512],
                                     start=False, stop=False)
                    nc.tensor.matmul(a_ps[:], idb_sb[:], nkt, start=False, stop=True)
                    b_ps = ps.tile([D, 512], F32, tag="b")
                    nc.tensor.matmul(b_ps[:], posw2_sb[:], pe1r[:], start=True, stop=False)
                    nc.tensor.matmul(b_ps[:], idb_sb[:], vt, start=False, stop=True)

                    y_sb = mn.tile([D, 512], BF16, tag="ysb")
                    nc.scalar.copy(y_sb[:], a_ps[:])

                    h1r = mn.tile([D, 4 * 512], BF16, tag="h1r")
                    for j in range(4):
                        h1_ps = ps.tile([D, 512], F32, tag="h1")
                        nc.tensor.matmul(h1_ps[:], a1w_sb[:, j * P:(j + 1) * P],
                                         y_sb[:], start=True, stop=True)
                        dst = h1r[:, j * 512:(j + 1) * 512]
                        if j < 3:
                            nc.scalar.activation(dst, h1_ps[:],
                                                 mybir.ActivationFunctionType.Relu,
                                                 bias=a1b_sb[:, j:j + 1])
                        else:
                            # a1 bias is zero in this model configuration
                            nc.vector.tensor_scalar_max(dst, h1_ps[:], 0.0)

                    h2_ps = ps.tile([D, 512], F32, tag="pe1h2")
                    for j in range(4):
                        nc.tensor.matmul(h2_ps[:], a2w_sb[:, j * P:(j + 1) * P],
                                         h1r[:, j * 512:(j + 1) * 512],
                                         start=(j == 0), stop=(j == 3))
                    # ez = [exp(h2+b) | exp*(v+pe)] so one segmented reduce
                    # yields both softmax sums
                    ez = mn.tile([D, 1024], F32, tag="ez")
                    nc.scalar.activation(ez[:, 0:512], h2_ps[:],
                                         mybir.ActivationFunctionType.Exp,
                                         bias=b2s_sb[:, 0:1])
                    nc.vector.tensor_mul(ez[:, 512:1024], ez[:, 0:512], b_ps[:])

                    uz = kn.tile([P, 64], F32, tag="uz")
                    nc.vector.reduce_sum(uz[:],
                                         ez[:].rearrange("p (a b) -> p a b", b=KNN),
                                         axis=mybir.AxisListType.X)
                    zr = kn.tile([P, 32], F32, tag="zr")
                    nc.vector.reciprocal(zr[:], uz[:, 0:32])
                    nc.vector.tensor_mul(aggt[:, r * 32:(r + 1) * 32],
                                         uz[:, 32:64], zr[:])

                def out_tile(t, aggt):
                    o_ps = ps.tile([P, 512], F32, tag="asp")
                    nc.tensor.matmul(o_ps[:, 0:D], aggt[:], wout_sb[:],
                                     start=True, stop=True)
                    o_sb = mn.tile([P, D], F32, tag="osb")
                    nc.vector.tensor_add(o_sb[:], o_ps[:, 0:D],
                                         ori_sb[:, t * D:(t + 1) * D])
                    nc.sync.dma_start(out=y_out[t * P:(t + 1) * P, :], in_=o_sb[:])

                # software pipeline, two tiles deep. During tile t:
                #  - step5 for t+1 (index transpose) runs first -- its scan
                #    inputs completed during t-1, so the in-order PE queue
                #    never stalls on it
                #  - tile t+1's gathers prefetch, one per group
                #  - tile t+2's scores + DVE scans fill ACT/DVE slack
                # so the PE rolls straight from tile t into t+1 and HAM
                # stays un-throttled.
                SCANS = {0: (0, 1), 1: (2,), 2: (3,), 3: (4,)}
                knn_alloc(0)
                knn_step(0, 0)
                if pt >= 12:
                    # x-projection setup overlaps tile 0/1's knn scans
                    sched = {3: (0, 1), 5: (0, 2), 7: (0, 3), 9: (0, 4),
                             11: (0, 5), 13: (1, 1), 14: (1, 2), 15: (1, 3)}
                    for t in range(pt):
                        setup_x_tile(t)
                        if t == 11:
                            knn_alloc(1)
                            knn_step(1, 0)
                        if t in sched:
                            knn_step(*sched[t])
                    knn_step(1, 4)
                else:
                    for t in range(pt):
                        setup_x_tile(t)
                    for k in range(1, 6):
                        knn_step(0, k)
                    if pt > 1:
                        knn_alloc(1)
                        for k in range(5):
                            knn_step(1, k)
                gts_map = {0: emit_gathers(0)}
                for t in range(pt):
                    aggt = aggp.tile([P, P], BF16, tag="aggt", name=f"aggt{t}")
                    if t + 2 < pt:
                        knn_alloc(t + 2)
                    gts = gts_map.pop(t)
                    if t + 1 < pt:
                        gts_map[t + 1] = []
                    for gi in range(4):
                        if gi == 0 and 0 < t + 1 < pt:
                            knn_step(t + 1, 5)
                        if t + 1 < pt:
                            gts_map[t + 1].append(emit_gather(t + 1, gi))
                        g = 4 * t + gi
                        gt = gts[gi]
                        mlp_group(t, g, gt[:, 0, :], gt[:, 1, :],
                                  gt[0:64, 2, :], aggt)
                        # t+2's scans go AFTER the group's DVE ops so the g3
                        # softmax tail (which gates out_tile) is never queued
                        # behind a 2.3us scan on the in-order DVE
                        if t + 2 < pt:
                            for k in SCANS[gi]:
                                knn_step(t + 2, k)
                    out_tile(t, aggt)

    nc.compile()
    return nc


def _prep_consts(w_in, w_qkv, w_out,
                 pos_w1, pos_b1, pos_bn_g, pos_bn_b, pos_bn_m, pos_bn_v,
                 pos_w2, pos_b2,
                 attn_w1, attn_b1, attn_bn_g, attn_bn_b, attn_bn_m, attn_bn_v,
                 attn_w2, attn_b2):
    bf = _bf()
    f = np.float32

    wqkv_eff = (w_in.astype(f) @ w_qkv.astype(f)).astype(bf)

    s_p = (pos_bn_g / np.sqrt(pos_bn_v + EPS)).astype(f)
    w1p = (pos_w1 * s_p[None, :]).astype(f)
    b1p = ((pos_b1 - pos_bn_m) * s_p + pos_bn_b).astype(f)

    s_a = (attn_bn_g / np.sqrt(attn_bn_v + EPS)).astype(f)
    a1 = (attn_w1 * s_a[None, :]).astype(f)
    a1b = ((attn_b1 - attn_bn_m) * s_a + attn_bn_b).astype(f)
    assert np.all(a1b[3 * P:] == 0.0), "kernel fast path assumes zero bias on h1[3]"

    a2 = (attn_w2.astype(f) * f(SCALE))
    a2resh = np.concatenate([a2[j * P:(j + 1) * P, :] for j in range(4)], axis=1)
    b2s = (attn_b2.astype(f) * f(SCALE))
    assert np.all(pos_b2 == 0.0), "kernel assumes zero pos_b2"

    # rep[:, r*512 + p] selects row 32r + p//16 — replicates the r-th 32-point
    # chunk of a 128-point tile across its 16 neighbor slots.
    rep = np.zeros((P, 4 * 512), f)
    for r in range(4):
        cols = r * 512 + np.arange(512)
        rep[32 * r + np.arange(512) // KNN, cols] = 1.0

    return dict(
        wqkv=wqkv_eff,
        w1p=w1p.astype(bf),
        posw2=pos_w2.astype(bf),
        a1w=a1.astype(bf),
        a2w=a2resh.astype(bf),
        wout=w_out.astype(bf),
        rep32=rep.astype(bf),
        identb=np.eye(P, dtype=f).astype(bf),
        identf=np.eye(P, dtype=f),
        identh=np.eye(P, dtype=np.float16),
        ones3=np.ones((3, 1), f),
        b1p=b1p.reshape(POS_H, 1),
        a1b=np.ascontiguousarray(a1b.reshape(4, P).T),
        b2s=b2s.reshape(P, 1),
    )


def kernel(**inputs) -> np.ndarray:
    ori_x = np.asarray(inputs["ori_x"], np.float32)
    pos = np.asarray(inputs["pos"], np.float32)
    b, n, _ = ori_x.shape

    consts = _prep_consts(
        **{k: np.asarray(v, np.float32) for k, v in inputs.items()
           if k not in ("ori_x", "pos")})

    nc = build_bass(n)
    in_maps = []
    for c in range(b):
        m = dict(consts)
        m["x_in"] = np.ascontiguousarray(ori_x[c])
        m["pos_in"] = np.ascontiguousarray(pos[c])
        in_maps.append(m)

    res = run_bass_kernel_spmd(nc, in_maps, list(range(b)))
    out = np.stack([np.asarray(res.results[c]["y_out"]) for c in range(b)], axis=0)
    return out.astype(np.float32)


if __name__ == "__main__":
    print("smoke build only")
    build_bass(512)
    print("built OK")
